# revision 1
# baseline (speedup 1.0000x reference)
"""CAMMambaBlock Trainium2 kernel.

Data-parallel over batch: 8 batch elements -> 8 NeuronCores. Each core runs
the full block (LayerNorm -> in_proj -> causal depthwise conv -> SiLU ->
x_proj -> dt softplus -> selective scan -> gating -> out_proj -> residual)
on its own (c=128, L=9216) slice, streaming over L in 6 chunks of 1536.

Engine assignment (chosen from measured TRN2 rates):
- DVE: the 16 per-state selective scans (feedback-bound, DVE-only,
  ~2.1 cyc/step) plus pure-bf16 tensor_tensor mults (2x mode) for
  u = v*B and p = h*C.
- Scalar: all activations (exp/ln softplus + silu chains), PSUM->bf16
  copies, the 16 dA = exp(A_n*dt) tiles (precomputed into a 13-deep ring
  so Scalar drains early and can run chunk k+1's prefix during chunk k's
  scans).
- PE: all matmuls; the 16-state readout sum as identity-matmul PSUM
  accumulation (replaces 15 elementwise adds); the residual add via an
  f32 identity matmul.
- Pool: carry extraction (bf16->f32 column copies) + SWDGE casts.
- DMA: B/C row broadcast via DRAM bounce (bf16); state-0 B pieces are
  written per-sub so the first scan of each chunk starts early.
Chunk k's readout (gate + out_proj + residual) is deferred into chunk
k+1's scan section so its ready-to-run DVE ops fill the prefix-latency
bubble at chunk boundaries.
"""
import types
import numpy as np
import ml_dtypes
from contextlib import ExitStack

import bass_rust

import concourse.bass as bass
import concourse.bacc as bacc
import concourse.tile as tile
from concourse import mybir
from concourse.bass_utils import run_bass_kernel_spmd
from concourse.hw_specs import get_activation_tables


def _single_act_table(self):
    """Force every activation onto natural_log_exp_and_others so the
    table-load pass hoists to one load."""
    if not any(i.opcode == "Activation" for i in self.all_instructions()):
        return
    keep = "natural_log_exp_and_others"
    tables = [(n, (f if n == keep else set()))
              for n, f in get_activation_tables(self.m.arch).items()]
    bass_rust.insert_act_table_loads(self, tables)

F32 = mybir.dt.float32
BF16 = mybir.dt.bfloat16
AF = mybir.ActivationFunctionType
OP = mybir.AluOpType

C = 128           # channels == d_inner == partitions
NSTATE = 16       # SSM state dim
RANK = 8          # dt rank
LN_EPS = 1e-5
DCONV = 4

L_FULL = 96 * 96  # 9216


def build_nc(L, Tc, sub=512):
    assert L % Tc == 0 and Tc % sub == 0
    nchunk = L // Tc
    nsub = Tc // sub

    nc = bacc.Bacc()
    x_in = nc.declare_dram_parameter("x", [C, L], F32, isOutput=False)
    w_inT = nc.declare_dram_parameter("w_inT", [C, 5 * C], BF16, isOutput=False)
    w_xpT = nc.declare_dram_parameter("w_xpT", [C, RANK + 2 * NSTATE], BF16,
                                      isOutput=False)
    w_dtT = nc.declare_dram_parameter("w_dtT", [RANK, C], BF16, isOutput=False)
    w_outT = nc.declare_dram_parameter("w_outT", [C, C], BF16, isOutput=False)
    ident = nc.declare_dram_parameter("ident", [C, C], BF16, isOutput=False)
    diag_d_in = nc.declare_dram_parameter("diag_d", [C, C], BF16,
                                          isOutput=False)
    identf = nc.declare_dram_parameter("identf", [C, C], F32, isOutput=False)
    # per-partition columns:
    # [ln_w, ln_b, conv_b, dt_b, D, unused*4, eps, -conv_b]
    cols = nc.declare_dram_parameter("cols", [C, 11], F32, isOutput=False)
    a_cols = nc.declare_dram_parameter("a_cols", [C, NSTATE], F32,
                                       isOutput=False)
    y_out = nc.declare_dram_parameter("y", [C, L], F32, isOutput=True)

    with tile.TileContext(nc) as tc, ExitStack() as ctx:
        wpool = ctx.enter_context(tc.tile_pool(name="weights", bufs=1))
        state = ctx.enter_context(tc.tile_pool(name="state", bufs=1))
        io = ctx.enter_context(tc.tile_pool(name="io", bufs=2))
        gate3 = ctx.enter_context(tc.tile_pool(name="gate3", bufs=3))
        work = ctx.enter_context(tc.tile_pool(name="work", bufs=2))
        scanp = ctx.enter_context(tc.tile_pool(name="scan", bufs=3))
        bcrp = ctx.enter_context(tc.tile_pool(name="bcrp", bufs=2))
        sqp = ctx.enter_context(tc.tile_pool(name="sqp", bufs=1))
        scr = ctx.enter_context(tc.tile_pool(name="scratch", bufs=2))
        dram = ctx.enter_context(tc.tile_pool(name="dram", bufs=2,
                                              space="DRAM"))
        ps_st = ctx.enter_context(tc.tile_pool(name="ps_st", bufs=1,
                                               space="PSUM"))
        ps_a = ctx.enter_context(tc.tile_pool(name="ps_a", bufs=1,
                                              space="PSUM"))
        ps_b = ctx.enter_context(tc.tile_pool(name="ps_b", bufs=2,
                                              space="PSUM"))
        ps_y = ctx.enter_context(tc.tile_pool(name="ps_y", bufs=1,
                                              space="PSUM"))

        # ---- chunk-0 inputs first: nothing depends on weights for the
        # LN stats, and the weight DMAs would otherwise delay them ----
        ones_c = wpool.tile([C, C], BF16, tag="ones")
        nc.gpsimd.memset(ones_c[:], 1.0 / C)
        xin0 = io.tile([C, Tc], F32, tag="xin", name="xin")
        nc.sync.dma_start(xin0[:], x_in[:, 0:Tc])
        xin_bf0 = io.tile([C, Tc], BF16, tag="xinbf", name="xinbf")
        nc.gpsimd.dma_start(xin_bf0[:], x_in[:, 0:Tc])
        sq0 = sqp.tile([C, Tc], BF16, tag="sq", name="sq")
        nc.scalar.activation(sq0[:], xin0[:], AF.Square)
        mus0, m2s0 = [], []
        for j in range(nsub):
            sl = slice(j * sub, (j + 1) * sub)
            mu = ps_st.tile([C, sub], F32, tag="mu", name="mu")
            nc.tensor.matmul(mu[:], ones_c[:], xin_bf0[:, sl],
                             start=True, stop=True)
            m2 = ps_st.tile([C, sub], F32, tag="m2", name="m2")
            nc.tensor.matmul(m2[:], ones_c[:], sq0[:, sl],
                             start=True, stop=True)
            mus0.append(mu)
            m2s0.append(m2)
        pre0 = (xin0, xin_bf0, mus0, m2s0)

        # ---- weights to SBUF (once) ----
        winT = wpool.tile([C, 5 * C], BF16, tag="winT")
        nc.sync.dma_start(winT[:], w_inT[:])
        wxpT = wpool.tile([C, RANK + 2 * NSTATE], BF16, tag="wxpT")
        nc.sync.dma_start(wxpT[:], w_xpT[:])
        wdtT = wpool.tile([RANK, C], BF16, tag="wdtT")
        nc.sync.dma_start(wdtT[:], w_dtT[:])
        woutT = wpool.tile([C, C], BF16, tag="woutT")
        nc.sync.dma_start(woutT[:], w_outT[:])
        idn = wpool.tile([C, C], BF16, tag="idn")
        nc.sync.dma_start(idn[:], ident[:])
        diag_d = wpool.tile([C, C], BF16, tag="diag_d")
        nc.sync.dma_start(diag_d[:], diag_d_in[:])
        idnf = wpool.tile([C, C], F32, tag="idnf")
        nc.sync.dma_start(idnf[:], identf[:])
        colsb = wpool.tile([C, 11], F32, tag="cols")
        nc.sync.dma_start(colsb[:], cols[:])
        acol = wpool.tile([C, NSTATE], F32, tag="acol")
        nc.sync.dma_start(acol[:], a_cols[:])
        ln_w, ln_b = colsb[:, 0:1], colsb[:, 1:2]
        conv_b, dt_b, d_col = colsb[:, 2:3], colsb[:, 3:4], colsb[:, 4:5]
        eps_col, ncb = colsb[:, 9:10], colsb[:, 10:11]

        # ---- persistent carry state (f32 columns) ----
        carries = []
        for n in range(NSTATE):
            t = state.tile([C, 1], F32, tag=f"carry{n}")
            carries.append(t)

        # ---- streaming loop ----
        # dA for all 16 states is precomputed right after dt (dapool), so the
        # scalar engine is free to run chunk k+1's prefix during the late
        # scans of chunk k. Carries go via Pool to keep Scalar unblocked.
        dapool = ctx.enter_context(tc.tile_pool(name="dapool", bufs=13))
        prev_un_ref = [None]

        def prefix(k, pre):
            t0 = k * Tc
            if pre is None:
                xin = io.tile([C, Tc], F32, tag="xin", name="xin")
                nc.sync.dma_start(xin[:], x_in[:, t0:t0 + Tc])
                xin_bf = io.tile([C, Tc], BF16, tag="xinbf", name="xinbf")
                nc.gpsimd.dma_start(xin_bf[:], x_in[:, t0:t0 + Tc])
                sq = sqp.tile([C, Tc], BF16, tag="sq", name="sq")
                nc.scalar.activation(sq[:], xin[:], AF.Square)
                mus, m2s = [], []
                for j in range(nsub):
                    sl = slice(j * sub, (j + 1) * sub)
                    mu = ps_st.tile([C, sub], F32, tag="mu", name="mu")
                    nc.tensor.matmul(mu[:], ones_c[:], xin_bf[:, sl],
                                     start=True, stop=True)
                    m2 = ps_st.tile([C, sub], F32, tag="m2", name="m2")
                    nc.tensor.matmul(m2[:], ones_c[:], sq[:, sl],
                                     start=True, stop=True)
                    mus.append(mu)
                    m2s.append(m2)
            else:
                xin, xin_bf, mus, m2s = pre

            un = work.tile([C, Tc + DCONV - 1], BF16, tag="un", name="un")
            if k == 0:
                nc.vector.memset(un[:, 0:DCONV - 1], 0.0)
            else:
                nc.vector.tensor_copy(un[:, 0:DCONV - 1],
                                      prev_un_ref[0][:, Tc:Tc + DCONV - 1])
            prev_un_ref[0] = un
            for j in range(nsub):
                sl = slice(j * sub, (j + 1) * sub)
                mu, m2 = mus[j], m2s[j]
                musq = scr.tile([C, sub], F32, tag="musq", name="musq")
                nc.scalar.activation(musq[:], mu[:], AF.Square)
                var = scr.tile([C, sub], F32, tag="var", name="var")
                nc.vector.tensor_tensor(var[:], m2[:], musq[:], OP.subtract)
                lnv = scr.tile([C, sub], F32, tag="lnv", name="lnv")
                nc.scalar.activation(lnv[:], var[:], AF.Ln, bias=eps_col)
                rstd = scr.tile([C, sub], BF16, tag="rstd", name="rstd")
                nc.scalar.activation(rstd[:], lnv[:], AF.Exp, scale=-0.5)
                mub = scr.tile([C, sub], BF16, tag="mub", name="mub")
                nc.scalar.copy(mub[:], mu[:])
                dmu = scr.tile([C, sub], BF16, tag="dmu", name="dmu")
                nc.vector.tensor_tensor(dmu[:], xin_bf[:, sl], mub[:],
                                        OP.subtract)
                dr = scr.tile([C, sub], BF16, tag="dr", name="dr")
                nc.vector.tensor_tensor(dr[:], dmu[:], rstd[:], OP.mult)
                nc.vector.tensor_scalar(
                    un[:, DCONV - 1 + j * sub:DCONV - 1 + (j + 1) * sub],
                    dr[:], ln_w, ln_b, OP.mult, OP.add)

            zs = gate3.tile([C, Tc], BF16, tag="zs", name="zs")
            xs = gate3.tile([C, Tc], BF16, tag="xs", name="xs")
            for j in range(nsub):
                sl = slice(j * sub, (j + 1) * sub)
                xm_ps = ps_a.tile([C, sub], F32, tag="xm_ps", name="xm_ps")
                for kk in range(DCONV):
                    nc.tensor.matmul(
                        xm_ps[:], winT[:, kk * C:(kk + 1) * C],
                        un[:, kk + j * sub:kk + j * sub + sub],
                        start=(kk == 0), stop=(kk == DCONV - 1))
                ec1 = scr.tile([C, sub], F32, tag="ec1", name="ec1")
                nc.scalar.activation(ec1[:], xm_ps[:], AF.Exp, scale=-1.0,
                                     bias=ncb)
                ec2 = scr.tile([C, sub], F32, tag="ec2", name="ec2")
                nc.scalar.activation(ec2[:], ec1[:], AF.Ln, bias=1.0)
                sgc = scr.tile([C, sub], BF16, tag="sgc", name="sgc")
                nc.scalar.activation(sgc[:], ec2[:], AF.Exp, scale=-1.0)
                xmb = scr.tile([C, sub], BF16, tag="xmb", name="xmb")
                nc.scalar.activation(xmb[:], xm_ps[:], AF.Identity,
                                     bias=conv_b)
                nc.vector.tensor_tensor(xs[:, sl], xmb[:], sgc[:], OP.mult)

            # layout: rows 0..7 = dtr, rows 8..39 = B/C (dtr first so the
            # dt matmul reads at base partition 0)
            bcdt = work.tile([2 * NSTATE + RANK, Tc], BF16, tag="bcdt",
                             name="bcdt")
            for j in range(nsub):
                sl = slice(j * sub, (j + 1) * sub)
                dblt = ps_b.tile([C, sub], F32, tag="mmb", name="dblt")
                dbl = dblt[0:RANK + 2 * NSTATE, :]
                nc.tensor.matmul(dbl, wxpT[:], xs[:, sl],
                                 start=True, stop=True)
                nc.scalar.copy(bcdt[:, sl],
                               dblt[0:2 * NSTATE + RANK, :])
            bcd = dram.tile([NSTATE, 2 * Tc], BF16, tag="bcd", name="bcd")
            # state-0 B pieces land first so the first scan can start early
            for j in range(nsub):
                sl = slice(j * sub, (j + 1) * sub)
                nc.sync.dma_start(bcd[0:1, j * sub:(j + 1) * sub],
                                  bcdt[RANK:RANK + 1, sl])
            nc.sync.dma_start(bcd[0:1, Tc:2 * Tc],
                              bcdt[RANK + 1:RANK + 2, :])
            nc.sync.dma_start(bcd[1:NSTATE, :],
                              bcdt[RANK + 2:RANK + 2 * NSTATE, :])

            dt_bf = work.tile([C, Tc], BF16, tag="dt", name="dt")
            for j in range(nsub):
                sl = slice(j * sub, (j + 1) * sub)
                dt_ps = ps_b.tile([C, sub], F32, tag="mmb", name="dt_ps")
                nc.tensor.matmul(dt_ps[:], wdtT[:], bcdt[0:RANK, sl],
                                 start=True, stop=True)
                spe = scr.tile([C, sub], F32, tag="spe", name="spe")
                nc.scalar.activation(spe[:], dt_ps[:], AF.Exp, bias=dt_b)
                nc.scalar.activation(dt_bf[:, sl], spe[:], AF.Ln, bias=1.0)

            vhA = work.tile([C, 2 * Tc], BF16, tag="vhA", name="vhA",
                            bufs=1)
            vhB = work.tile([C, 2 * Tc], BF16, tag="vhB", name="vhB",
                            bufs=1)
            for j in range(nsub):
                sl = slice(j * sub, (j + 1) * sub)
                nc.vector.tensor_tensor(
                    vhA[:, Tc + j * sub:Tc + (j + 1) * sub],
                    dt_bf[:, sl], xs[:, sl], OP.mult)
            nc.sync.dma_start(vhB[:, Tc:2 * Tc], vhA[:, Tc:2 * Tc])

            # dA for state 0 per sub (releases the first scan early), then
            # the other 15 full-chunk
            dAs = [None]
            dA0 = dapool.tile([C, Tc], BF16, tag="dA", name="dA")
            for j in range(nsub):
                sl = slice(j * sub, (j + 1) * sub)
                nc.scalar.activation(dA0[:, sl], dt_bf[:, sl], AF.Exp,
                                     scale=acol[:, 0:1])
            dAs[0] = dA0
            for n in range(1, NSTATE):
                dA = dapool.tile([C, Tc], BF16, tag="dA", name="dA")
                nc.scalar.activation(dA[:], dt_bf[:], AF.Exp,
                                     scale=acol[:, n:n + 1])
                dAs.append(dA)
            # z gate (needed only at readout) computed after the dA block
            for j in range(nsub):
                sl = slice(j * sub, (j + 1) * sub)
                z_ps = ps_b.tile([C, sub], F32, tag="mmb", name="z_ps")
                nc.tensor.matmul(z_ps[:], winT[:, 4 * C:5 * C],
                                 un[:, DCONV - 1 + j * sub:
                                     DCONV - 1 + j * sub + sub],
                                 start=True, stop=True)
                es1 = scr.tile([C, sub], F32, tag="es1", name="es1")
                nc.scalar.activation(es1[:], z_ps[:], AF.Exp, scale=-1.0)
                es2 = scr.tile([C, sub], F32, tag="es2", name="es2")
                nc.scalar.activation(es2[:], es1[:], AF.Ln, bias=1.0)
                sgz = scr.tile([C, sub], BF16, tag="sgz", name="sgz")
                nc.scalar.activation(sgz[:], es2[:], AF.Exp, scale=-1.0)
                zb = scr.tile([C, sub], BF16, tag="zb", name="zb")
                nc.scalar.copy(zb[:], z_ps[:])
                nc.vector.tensor_tensor(zs[:, sl], zb[:], sgz[:], OP.mult)
            return dict(xin=xin, zs=zs, xs=xs, vhA=vhA, vhB=vhB, bcd=bcd, dAs=dAs)

        def readout(k, P, ysum):
            t0 = k * Tc
            zs, xs = P["zs"], P["xs"]
            for j in range(nsub):
                sl = slice(j * sub, (j + 1) * sub)
                xrl = io.tile([C, sub], F32, tag="xrl", name="xrl")
                nc.sync.dma_start(xrl[:],
                                  x_in[:, t0 + j * sub:t0 + (j + 1) * sub])
                yg = scr.tile([C, sub], BF16, tag="yg", name="yg")
                nc.vector.tensor_tensor(yg[:], ysum[j], zs[:, sl], OP.mult)
                o_ps = ps_b.tile([C, sub], F32, tag="mmb", name="o_ps")
                nc.tensor.matmul(o_ps[:], woutT[:], yg[:],
                                 start=True, stop=False)
                nc.tensor.matmul(o_ps[:], idnf[:], xrl[:],
                                 start=False, stop=True)
                ob = io.tile([C, sub], F32, tag="ob", name="ob")
                nc.scalar.copy(ob[:], o_ps[:])
                nc.sync.dma_start(y_out[:, t0 + j * sub:t0 + (j + 1) * sub],
                                  ob[:])

        def scansec(k, P, prev_ro):
            t0 = k * Tc
            vhA, vhB, bcd = P["vhA"], P["vhB"], P["bcd"]
            dAs = P["dAs"]
            bcdf = bcd.tensor.reshape([1, NSTATE * 2 * Tc])
            pre = None
            if k + 1 < nchunk:
                t1 = (k + 1) * Tc
                nxin = io.tile([C, Tc], F32, tag="xin", name="xin")
                nc.sync.dma_start(nxin[:], x_in[:, t1:t1 + Tc])
                nxin_bf = io.tile([C, Tc], BF16, tag="xinbf", name="xinbf")
                nc.gpsimd.dma_start(nxin_bf[:], x_in[:, t1:t1 + Tc])
            ysum_t = ps_y.tile([C, Tc], F32, tag="ysum", name="ysum")
            ysum = [ysum_t[:, j * sub:(j + 1) * sub] for j in range(nsub)]
            for n in range(NSTATE):
                if n == 8 and k + 1 < nchunk:
                    nsq = sqp.tile([C, Tc], BF16, tag="sq", name="sq")
                    nc.scalar.activation(nsq[:], nxin[:], AF.Square)
                if n == 10 and k + 1 < nchunk:
                    nmus, nm2s = [], []
                    for j in range(nsub):
                        sl = slice(j * sub, (j + 1) * sub)
                        mu = ps_st.tile([C, sub], F32, tag="mu", name="mu")
                        nc.tensor.matmul(mu[:], ones_c[:], nxin_bf[:, sl],
                                         start=True, stop=True)
                        m2 = ps_st.tile([C, sub], F32, tag="m2", name="m2")
                        nc.tensor.matmul(m2[:], ones_c[:], nsq[:, sl],
                                         start=True, stop=True)
                        nmus.append(mu)
                        nm2s.append(m2)
                    pre = (nxin, nxin_bf, nmus, nm2s)
                dA = dAs[n]
                init = 0.0 if k == 0 else carries[n][:]
                vh = vhA if n % 2 == 0 else vhB
                if n == 0:
                    bcr0 = bcrp.tile([C, Tc], BF16, tag="bcr0",
                                     name="bcr0", bufs=1)
                    u0 = scanp.tile([C, Tc], BF16, tag="u0", name="u0",
                                    bufs=1)
                    for j in range(nsub):
                        sl = slice(j * sub, (j + 1) * sub)
                        nc.sync.dma_start(
                            bcr0[:, sl],
                            bcd[0:1, sl].broadcast_to([C, sub]))
                        nc.vector.tensor_tensor(
                            u0[:, sl],
                            vhA[:, Tc + j * sub:Tc + (j + 1) * sub],
                            bcr0[:, sl], OP.mult)
                        nc.vector.tensor_tensor_scan(
                            vh[:, sl], dA[:, sl], u0[:, sl],
                            init if j == 0 else vh[:, j * sub - 1:j * sub],
                            OP.mult, OP.add)
                else:
                    nc.vector.tensor_tensor_scan(
                        vh[:, 0:Tc], dA[:], po[:, Tc:2 * Tc], init,
                        OP.mult, OP.add)
                nc.gpsimd.tensor_copy(carries[n][:], vh[:, Tc - 1:Tc])
                # fused [p_n | u_{n+1}] = [h_n | v] * [C_n | B_{n+1}]:
                # in bcd, row n's C half abuts row n+1's B half, so the
                # coefficient pair is one contiguous flat slice
                po = scanp.tile([C, 2 * Tc], BF16, tag="po", name="po",
                                bufs=2)
                if n < NSTATE - 1:
                    bcr = bcrp.tile([C, 2 * Tc], BF16, tag="bcr",
                                    name="bcr")
                    nc.sync.dma_start(
                        bcr[:],
                        bcdf[0:1, (2 * n + 1) * Tc:(2 * n + 3) * Tc]
                        .broadcast_to([C, 2 * Tc]))
                    nc.vector.tensor_tensor(po[:], vh[:], bcr[:], OP.mult)
                else:
                    bcr = bcrp.tile([C, 2 * Tc], BF16, tag="bcr",
                                    name="bcr")
                    nc.sync.dma_start(
                        bcr[:, 0:Tc],
                        bcdf[0:1, (2 * n + 1) * Tc:(2 * n + 2) * Tc]
                        .broadcast_to([C, Tc]))
                    nc.vector.tensor_tensor(po[:, 0:Tc], vh[:, 0:Tc],
                                            bcr[:, 0:Tc], OP.mult)
                if n == 0 and prev_ro is not None:
                    # deferred readout of chunk k-1: emitted before the first
                    # write of this chunk's ysum so the WAR dep is tracked
                    readout(*prev_ro)
                for j in range(nsub):
                    nc.tensor.matmul(ysum[j], idn[:],
                                     po[:, j * sub:(j + 1) * sub],
                                     start=(n == 0), stop=False)

            for j in range(nsub):
                nc.tensor.matmul(ysum[j], diag_d[:],
                                 P["xs"][:, j * sub:(j + 1) * sub],
                                 start=False, stop=True)
            return pre, (k, P, ysum)

        pre = pre0
        prev_ro = None
        for k in range(nchunk):
            P = prefix(k, pre)
            pre, prev_ro = scansec(k, P, prev_ro)
        readout(*prev_ro)
    nc.insert_act_table_loads = types.MethodType(_single_act_table, nc)
    nc.compile()
    return nc


def prep_weights(ln_w, ln_b, in_proj_w, conv_w, conv_b, x_proj_w,
                 dt_proj_w, dt_proj_b, A_log, D, out_proj_w):
    eps = np.full_like(ln_w, LN_EPS)
    z = np.zeros_like(ln_w)
    cols = np.stack([ln_w, ln_b, conv_b, dt_proj_b, D,
                     z, z, z, z, eps, -conv_b], axis=1).astype(np.float32)
    return {
        "w_inT": np.ascontiguousarray(np.concatenate(
            [in_proj_w[:128].T * conv_w[:, kk][None, :]
             for kk in range(4)] + [in_proj_w[128:].T],
            axis=1).astype(ml_dtypes.bfloat16)),
        "w_xpT": np.ascontiguousarray(
            x_proj_w[list(range(8))
                     + [8 + (i // 2) + 16 * (i % 2) for i in range(32)]].T
            .astype(ml_dtypes.bfloat16)),
        "w_dtT": np.ascontiguousarray(dt_proj_w.T.astype(ml_dtypes.bfloat16)),
        "w_outT": np.ascontiguousarray(
            out_proj_w.T.astype(ml_dtypes.bfloat16)),
        "ident": np.eye(C, dtype=ml_dtypes.bfloat16),
        "diag_d": np.ascontiguousarray(
            np.diag(D).astype(ml_dtypes.bfloat16)),
        "identf": np.eye(C, dtype=np.float32),
        "cols": cols,
        "a_cols": np.ascontiguousarray(-np.exp(A_log.astype(np.float32))),
    }


def kernel(input, ln_w, ln_b, in_proj_w, conv_w, conv_b, x_proj_w,
           dt_proj_w, dt_proj_b, A_log, D, out_proj_w, _run=None):
    input = np.asarray(input, np.float32)
    b, c, H, W = input.shape
    L = H * W
    assert c == C and b == 8
    wts = prep_weights(
        np.asarray(ln_w, np.float32), np.asarray(ln_b, np.float32),
        np.asarray(in_proj_w, np.float32), np.asarray(conv_w, np.float32),
        np.asarray(conv_b, np.float32), np.asarray(x_proj_w, np.float32),
        np.asarray(dt_proj_w, np.float32), np.asarray(dt_proj_b, np.float32),
        np.asarray(A_log, np.float32), np.asarray(D, np.float32),
        np.asarray(out_proj_w, np.float32))
    nc = build_nc(L, 1536, 512)
    in_maps = []
    for i in range(8):
        m = {"x": np.ascontiguousarray(input[i].reshape(c, L))}
        m.update(wts)
        in_maps.append(m)
    run = _run or run_bass_kernel_spmd
    res = run(nc, in_maps, core_ids=list(range(8)))
    out = np.stack([np.asarray(res.results[i]["y"]).reshape(c, H, W)
                    for i in range(8)])
    return out.astype(np.float32)



# revision 2
# speedup vs baseline: 1.0089x; 1.0089x over previous
"""CAMMambaBlock Trainium2 kernel, v2.

Data-parallel over batch: 8 batch elements -> 8 NeuronCores. Each core runs
the full block on its (c=128, L=9216) slice, streaming over L in 6 chunks of
1536.

Key structure: the per-state mults are UNFUSED from the scan chain.
u_n = v*B_n depends only on prefix outputs, so all 16 scans run
back-to-back on DVE with no interleaved serial mults; p_n = h_n*C_n runs
after scan_n and feeds PE identity-matmul ysum accumulation. Silu/softplus
use direct activation functions (2 act-table swaps per chunk); ln_w/ln_b
are folded into the in_proj weights/biases host-side so LN is two DVE ops.
Cross-chunk software pipelining via emission hooks inside the scan loop.
"""
import types
import numpy as np
import ml_dtypes
from contextlib import ExitStack

import bass_rust

import concourse.bass as bass
import concourse.bacc as bacc
import concourse.tile as tile
from concourse import mybir
from concourse.bass_utils import run_bass_kernel_spmd
from concourse.hw_specs import get_activation_tables


def _two_act_tables(self):
    """Limit activation tables to the two we batch by, so the table-load
    pass inserts at most one swap per batch."""
    if not any(i.opcode == "Activation" for i in self.all_instructions()):
        return
    keep = ("natural_log_exp_and_others", "silu_and_others")
    tables = [(n, (f if n in keep else set()))
              for n, f in get_activation_tables(self.m.arch).items()]
    bass_rust.insert_act_table_loads(self, tables)

F32 = mybir.dt.float32
BF16 = mybir.dt.bfloat16
AF = mybir.ActivationFunctionType
OP = mybir.AluOpType

C = 128
NSTATE = 16
RANK = 8
LN_EPS = 1e-5
DCONV = 4

L_FULL = 96 * 96

# states whose p-mult runs on the Pool engine. Measured: Pool shares an SBUF
# port with DVE 2-port ops, so Pool compute during scans halves BOTH engines'
# throughput. Keep empty.
POOL_P = ()


def build_nc(L, Tc, sub=512):
    assert L % Tc == 0 and Tc % sub == 0
    nchunk = L // Tc
    nsub = Tc // sub

    nc = bacc.Bacc()
    x_in = nc.declare_dram_parameter("x", [C, L], F32, isOutput=False)
    w_inT = nc.declare_dram_parameter("w_inT", [C, 5 * C], BF16, isOutput=False)
    w_xpT = nc.declare_dram_parameter("w_xpT", [C, RANK + 2 * NSTATE], BF16,
                                      isOutput=False)
    w_dtT = nc.declare_dram_parameter("w_dtT", [RANK, C], BF16, isOutput=False)
    w_outT = nc.declare_dram_parameter("w_outT", [C, C], BF16, isOutput=False)
    ident = nc.declare_dram_parameter("ident", [C, C], BF16, isOutput=False)
    nident = nc.declare_dram_parameter("nident", [C, C], BF16, isOutput=False)
    diag_d_in = nc.declare_dram_parameter("diag_d", [C, C], BF16,
                                          isOutput=False)
    identf = nc.declare_dram_parameter("identf", [C, C], F32, isOutput=False)
    # per-partition columns:
    # [ln_w, ln_b, conv_b, dt_b, D, unused*4, eps, -conv_b]
    cols = nc.declare_dram_parameter("cols", [C, 13], F32, isOutput=False)
    a_cols = nc.declare_dram_parameter("a_cols", [C, NSTATE], F32,
                                       isOutput=False)
    y_out = nc.declare_dram_parameter("y", [C, L], F32, isOutput=True)

    with tile.TileContext(nc) as tc, ExitStack() as ctx:
        wpool = ctx.enter_context(tc.tile_pool(name="weights", bufs=1))
        io = ctx.enter_context(tc.tile_pool(name="io", bufs=2))
        sqp = ctx.enter_context(tc.tile_pool(name="sqp", bufs=1))
        lnp = ctx.enter_context(tc.tile_pool(name="lnp", bufs=1))
        unp = ctx.enter_context(tc.tile_pool(name="unp", bufs=2))
        gate = ctx.enter_context(tc.tile_pool(name="gate", bufs=2))
        dtvp = ctx.enter_context(tc.tile_pool(name="dtvp", bufs=2))
        bctp = ctx.enter_context(tc.tile_pool(name="bctp", bufs=2))
        scr = ctx.enter_context(tc.tile_pool(name="scr", bufs=2))
        scr0 = ctx.enter_context(tc.tile_pool(name="scr0", bufs=1))
        dap = ctx.enter_context(tc.tile_pool(name="dap", bufs=9))
        bcrp = ctx.enter_context(tc.tile_pool(name="bcrp", bufs=6))
        b0p = ctx.enter_context(tc.tile_pool(name="b0p", bufs=2))
        up = ctx.enter_context(tc.tile_pool(name="up", bufs=4))
        hp = ctx.enter_context(tc.tile_pool(name="hp", bufs=3))
        pp = ctx.enter_context(tc.tile_pool(name="pp", bufs=3))
        ygp = ctx.enter_context(tc.tile_pool(name="ygp", bufs=2))
        state = ctx.enter_context(tc.tile_pool(name="state", bufs=1))
        dram = ctx.enter_context(tc.tile_pool(name="dram", bufs=2,
                                              space="DRAM"))
        ps_st = ctx.enter_context(tc.tile_pool(name="ps_st", bufs=1,
                                               space="PSUM"))
        ps_b = ctx.enter_context(tc.tile_pool(name="ps_b", bufs=2,
                                              space="PSUM"))
        ps_y = ctx.enter_context(tc.tile_pool(name="ps_y", bufs=1,
                                              space="PSUM"))

        # ---- chunk-0 input first: nothing else gates the LN stats ----
        xin0 = io.tile([C, Tc], F32, tag="xin", name="xin0")
        nc.sync.dma_start(xin0[:], x_in[:, 0:Tc])
        xinbf0 = io.tile([C, Tc], BF16, tag="xinbf", name="xinbf0")
        nc.gpsimd.dma_start(xinbf0[:], x_in[:, 0:Tc])
        P0 = {"xin": xin0, "xinbf": xinbf0}

        # ---- weights ----
        ones_c = wpool.tile([C, C], BF16, tag="ones")
        nc.gpsimd.memset(ones_c[:], 1.0 / C)
        winT = wpool.tile([C, 5 * C], BF16, tag="winT")
        nc.sync.dma_start(winT[:], w_inT[:])
        wxpT = wpool.tile([C, RANK + 2 * NSTATE], BF16, tag="wxpT")
        nc.sync.dma_start(wxpT[:], w_xpT[:])
        wdtT = wpool.tile([RANK, C], BF16, tag="wdtT")
        nc.sync.dma_start(wdtT[:], w_dtT[:])
        woutT = wpool.tile([C, C], BF16, tag="woutT")
        nc.sync.dma_start(woutT[:], w_outT[:])
        idn = wpool.tile([C, C], BF16, tag="idn")
        nc.sync.dma_start(idn[:], ident[:])
        nidn = wpool.tile([C, C], BF16, tag="nidn")
        nc.sync.dma_start(nidn[:], nident[:])
        diag_d = wpool.tile([C, C], BF16, tag="diag_d")
        nc.sync.dma_start(diag_d[:], diag_d_in[:])
        idnf = wpool.tile([C, C], F32, tag="idnf")
        nc.sync.dma_start(idnf[:], identf[:])
        colsb = wpool.tile([C, 13], F32, tag="cols")
        nc.sync.dma_start(colsb[:], cols[:])
        acol = wpool.tile([C, NSTATE], F32, tag="acol")
        nc.sync.dma_start(acol[:], a_cols[:])
        conv_b, dt_b = colsb[:, 2:3], colsb[:, 3:4]
        zb_col = colsb[:, 5:6]
        corr_cols = colsb[:, 6:9]
        eps_col = colsb[:, 9:10]
        ncb_col = colsb[:, 10:11]
        nzb_col = colsb[:, 11:12]

        carries = [state.tile([C, 1], BF16, tag=f"carry{n}",
                              name=f"carry{n}")
                   for n in range(NSTATE)]

        P = {}  # per-chunk produced tiles

        def emit_in_dma(k):
            t0 = k * Tc
            d = P.setdefault(k, {})
            d["xin"] = io.tile([C, Tc], F32, tag="xin", name="xin")
            nc.sync.dma_start(d["xin"][:], x_in[:, t0:t0 + Tc])
            d["xinbf"] = io.tile([C, Tc], BF16, tag="xinbf", name="xinbf")
            nc.gpsimd.dma_start(d["xinbf"][:], x_in[:, t0:t0 + Tc])

        def emit_ln_a(k):
            d = P[k]
            sq = sqp.tile([C, Tc], BF16, tag="sq", name="sq")
            nc.scalar.activation(sq[:], d["xin"][:], AF.Square)
            mub = lnp.tile([C, Tc], BF16, tag="mub", name="mub")
            rstd = lnp.tile([C, Tc], BF16, tag="rstd", name="rstd")
            for j in range(nsub):
                sl = slice(j * sub, (j + 1) * sub)
                mu = ps_st.tile([C, sub], F32, tag="mu", name="mu")
                nc.tensor.matmul(mu[:], ones_c[:], d["xinbf"][:, sl],
                                 start=True, stop=True)
                musq = scr.tile([C, sub], BF16, tag="musq", name="musq")
                nc.scalar.activation(musq[:], mu[:], AF.Square)
                nc.scalar.copy(mub[:, sl], mu[:])
                var = ps_st.tile([C, sub], F32, tag="m2", name="var")
                nc.tensor.matmul(var[:], ones_c[:], sq[:, sl],
                                 start=True, stop=False)
                nc.tensor.matmul(var[:], nidn[:], musq[:],
                                 start=False, stop=True)
                lnv = scr0.tile([C, sub], F32, tag="lnv", name="lnv")
                nc.scalar.activation(lnv[:], var[:], AF.Ln, bias=eps_col)
                nc.scalar.activation(rstd[:, sl], lnv[:], AF.Exp, scale=-0.5)
            d["mub"], d["rstd"] = mub, rstd

        def emit_ln_b(k):
            d = P[k]
            # ln_w is folded into the in_proj weights and ln_b into the
            # conv/z biases (host-side), so un is just (x-mu)*rstd. Halo
            # lives at cols 1..3; data at 4..Tc+3 (4B-aligned write).
            un = unp.tile([C, Tc + DCONV], BF16, tag="un", name="un")
            if k == 0:
                nc.vector.memset(un[:, 1:DCONV], 0.0)
            else:
                nc.vector.tensor_copy(un[:, 1:DCONV],
                                      P[k - 1]["un"][:, Tc + 1:Tc + DCONV])
            dmu = scr.tile([C, Tc], BF16, tag="dmu", name="dmu")
            nc.vector.tensor_tensor(dmu[:], d["xinbf"][:], d["mub"][:],
                                    OP.subtract)
            nc.vector.tensor_tensor(un[:, DCONV:Tc + DCONV],
                                    dmu[:], d["rstd"][:], OP.mult)
            d["un"] = un

        def emit_conv(k):
            d = P[k]
            un = d["un"]
            xs = gate.tile([C, Tc], BF16, tag="xs", name="xs")
            zs = gate.tile([C, Tc], BF16, tag="zs", name="zs")
            for j in range(nsub):
                sl = slice(j * sub, (j + 1) * sub)
                xm_ps = ps_b.tile([C, sub], F32, tag="mmb", name="xm_ps")
                for kk in range(DCONV):
                    nc.tensor.matmul(
                        xm_ps[:], winT[:, kk * C:(kk + 1) * C],
                        un[:, kk + 1 + j * sub:kk + 1 + j * sub + sub],
                        start=(kk == 0), stop=(kk == DCONV - 1))
                nc.scalar.activation(xs[:, sl], xm_ps[:], AF.Silu,
                                     bias=conv_b)
            for j in range(nsub):
                sl = slice(j * sub, (j + 1) * sub)
                z_ps = ps_b.tile([C, sub], F32, tag="mmb", name="z_ps")
                nc.tensor.matmul(z_ps[:], winT[:, 4 * C:5 * C],
                                 un[:, DCONV + j * sub:
                                     DCONV + j * sub + sub],
                                 start=True, stop=True)
                nc.scalar.activation(zs[:, sl], z_ps[:], AF.Silu,
                                     bias=zb_col)
            d["xs"], d["zs"] = xs, zs

        def emit_proj(k):
            d = P[k]
            xs = d["xs"]
            bcdt = bctp.tile([2 * NSTATE + RANK, Tc], BF16, tag="bcdt",
                             name="bcdt")
            for j in range(nsub):
                sl = slice(j * sub, (j + 1) * sub)
                dblt = ps_b.tile([C, sub], F32, tag="mmb", name="dblt")
                nc.tensor.matmul(dblt[0:RANK + 2 * NSTATE, :], wxpT[:],
                                 xs[:, sl], start=True, stop=True)
                nc.scalar.copy(bcdt[:, sl], dblt[0:2 * NSTATE + RANK, :])
            bcd = dram.tile([NSTATE, 2 * Tc], BF16, tag="bcd", name="bcd")
            # B_0 per-sub first so the first u-mult can start early
            for j in range(nsub):
                nc.sync.dma_start(bcd[0:1, j * sub:(j + 1) * sub],
                                  bcdt[RANK:RANK + 1, j * sub:(j + 1) * sub])
            nc.sync.dma_start(bcd[0:1, Tc:2 * Tc],
                              bcdt[RANK + 1:RANK + 2, :])
            nc.sync.dma_start(bcd[1:NSTATE, :],
                              bcdt[RANK + 2:RANK + 2 * NSTATE, :])
            d["bcd"] = bcd

            dt_bf = dtvp.tile([C, Tc], BF16, tag="dt", name="dt")
            for j in range(nsub):
                sl = slice(j * sub, (j + 1) * sub)
                dt_ps = ps_b.tile([C, sub], F32, tag="mmb", name="dt_ps")
                nc.tensor.matmul(dt_ps[:], wdtT[:], bcdt[0:RANK, sl],
                                 start=True, stop=True)
                spe = scr0.tile([C, sub], F32, tag="spe", name="spe")
                nc.scalar.activation(spe[:], dt_ps[:], AF.Exp, bias=dt_b)
                nc.scalar.activation(dt_bf[:, sl], spe[:], AF.Ln, bias=1.0)
            d["dt"] = dt_bf

        def emit_dA(k):
            d = P[k]
            dAs = []
            for n in range(NSTATE):
                dA = dap.tile([C, Tc], BF16, tag="dA", name="dA")
                nc.scalar.activation(dA[:], d["dt"][:], AF.Exp,
                                     scale=acol[:, n:n + 1])
                dAs.append(dA)
            d["dAs"] = dAs
            # broadcast DMAs: B_0, then pairs (C_n | B_{n+1}), then C_15
            bcd = d["bcd"]
            bcdf = bcd.tensor.reshape([1, NSTATE * 2 * Tc])
            b0 = b0p.tile([C, Tc], BF16, tag="b0", name="b0")
            for j in range(nsub):
                sl = slice(j * sub, (j + 1) * sub)
                nc.sync.dma_start(b0[:, sl],
                                  bcd[0:1, sl].broadcast_to([C, sub]))
            d["b0"] = b0
            prs = []
            for n in range(NSTATE - 1):
                pr = bcrp.tile([C, 2 * Tc], BF16, tag="bcr", name="bcr")
                nc.sync.dma_start(
                    pr[:],
                    bcdf[0:1, (2 * n + 1) * Tc:(2 * n + 3) * Tc]
                    .broadcast_to([C, 2 * Tc]))
                prs.append(pr)
            c15 = b0p.tile([C, Tc], BF16, tag="c15", name="c15")
            nc.sync.dma_start(
                c15[:],
                bcdf[0:1, (2 * NSTATE - 1) * Tc:2 * NSTATE * Tc]
                .broadcast_to([C, Tc]))
            d["prs"], d["c15"] = prs, c15

        def emit_v(k):
            d = P[k]
            v = dtvp.tile([C, Tc], BF16, tag="v", name="v")
            nc.vector.tensor_tensor(v[:], d["dt"][:], d["xs"][:], OP.mult)
            d["v"] = v
            d["us"] = [None] * NSTATE

        def emit_u(k, n):
            d = P[k]
            u = up.tile([C, Tc], BF16, tag="u", name="u")
            if n == 0:
                nc.vector.tensor_tensor(u[:], d["v"][:], d["b0"][:], OP.mult)
            else:
                nc.vector.tensor_tensor(u[:], d["v"][:],
                                        d["prs"][n - 1][:, Tc:2 * Tc],
                                        OP.mult)
            d["us"][n] = u

        def emit_readout(k):
            d = P[k]
            t0 = k * Tc
            ysum, zs, xin = d["ysum"], d["zs"], d["xin"]
            for j in range(nsub):
                sl = slice(j * sub, (j + 1) * sub)
                yc = ygp.tile([C, sub], BF16, tag="yc", name="yc")
                nc.scalar.copy(yc[:], ysum[:, sl])
                yg = ygp.tile([C, sub], BF16, tag="yg", name="yg")
                nc.vector.tensor_tensor(yg[:], yc[:], zs[:, sl], OP.mult)
                o_ps = ps_b.tile([C, sub], F32, tag="mmb", name="o_ps")
                nc.tensor.matmul(o_ps[:], woutT[:], yg[:],
                                 start=True, stop=False)
                nc.tensor.matmul(o_ps[:], idnf[:], xin[:, sl],
                                 start=False, stop=True)
                ob = ygp.tile([C, sub], F32, tag="ob", name="ob")
                nc.scalar.copy(ob[:], o_ps[:])
                nc.sync.dma_start(y_out[:, t0 + j * sub:t0 + (j + 1) * sub],
                                  ob[:])

        def emit_prefix0():
            """Chunk-0 prefix, per-sub pipelined so the first scan starts
            ~25us earlier. Silu synthesized from exp/ln (no table swap in
            the ramp). State-0's scan runs as chained sub-scans emitted
            inline; states 0/1 dA and u_0 are produced per-sub."""
            d = P[0]
            sq = sqp.tile([C, Tc], BF16, tag="sq", name="sq")
            mub = lnp.tile([C, Tc], BF16, tag="mub", name="mub")
            rstd = lnp.tile([C, Tc], BF16, tag="rstd", name="rstd")
            un = unp.tile([C, Tc + DCONV], BF16, tag="un", name="un")
            nc.vector.memset(un[:, 1:DCONV], 0.0)
            xs = gate.tile([C, Tc], BF16, tag="xs", name="xs")
            zs = gate.tile([C, Tc], BF16, tag="zs", name="zs")
            bcdt = bctp.tile([2 * NSTATE + RANK, Tc], BF16, tag="bcdt",
                             name="bcdt")
            bcd = dram.tile([NSTATE, 2 * Tc], BF16, tag="bcd", name="bcd")
            dt_bf = dtvp.tile([C, Tc], BF16, tag="dt", name="dt")
            v = dtvp.tile([C, Tc], BF16, tag="v", name="v")
            dA0 = dap.tile([C, Tc], BF16, tag="dA", name="dA0")
            dA1 = dap.tile([C, Tc], BF16, tag="dA", name="dA1")
            b0 = b0p.tile([C, Tc], BF16, tag="b0", name="b0")
            u0 = up.tile([C, Tc], BF16, tag="u", name="u0")
            h0 = hp.tile([C, Tc], BF16, tag="h", name="h0")
            for j in range(nsub):
                sl = slice(j * sub, (j + 1) * sub)
                nc.scalar.activation(sq[:, sl], d["xin"][:, sl], AF.Square)
                mu = ps_st.tile([C, sub], F32, tag="mu", name="mu")
                nc.tensor.matmul(mu[:], ones_c[:], d["xinbf"][:, sl],
                                 start=True, stop=True)
                musq = scr.tile([C, sub], BF16, tag="musq", name="musq")
                nc.scalar.activation(musq[:], mu[:], AF.Square)
                nc.scalar.copy(mub[:, sl], mu[:])
                var = ps_st.tile([C, sub], F32, tag="m2", name="var")
                nc.tensor.matmul(var[:], ones_c[:], sq[:, sl],
                                 start=True, stop=False)
                nc.tensor.matmul(var[:], nidn[:], musq[:],
                                 start=False, stop=True)
                lnv = scr0.tile([C, sub], F32, tag="lnv", name="lnv")
                nc.scalar.activation(lnv[:], var[:], AF.Ln, bias=eps_col)
                nc.scalar.activation(rstd[:, sl], lnv[:], AF.Exp, scale=-0.5)
                dmu = scr.tile([C, sub], BF16, tag="dmu", name="dmu")
                nc.vector.tensor_tensor(dmu[:], d["xinbf"][:, sl],
                                        mub[:, sl], OP.subtract)
                nc.vector.tensor_tensor(
                    un[:, DCONV + j * sub:DCONV + (j + 1) * sub],
                    dmu[:], rstd[:, sl], OP.mult)
                xm_ps = ps_b.tile([C, sub], F32, tag="mmb", name="xm_ps")
                for kk in range(DCONV):
                    nc.tensor.matmul(
                        xm_ps[:], winT[:, kk * C:(kk + 1) * C],
                        un[:, kk + 1 + j * sub:kk + 1 + j * sub + sub],
                        start=(kk == 0), stop=(kk == DCONV - 1))
                ec1 = scr0.tile([C, sub], F32, tag="ec1", name="ec1")
                nc.scalar.activation(ec1[:], xm_ps[:], AF.Exp, scale=-1.0,
                                     bias=ncb_col)
                ec2 = scr0.tile([C, sub], F32, tag="ec2", name="ec2")
                nc.scalar.activation(ec2[:], ec1[:], AF.Ln, bias=1.0)
                sgc = scr0.tile([C, sub], BF16, tag="sgc", name="sgc")
                nc.scalar.activation(sgc[:], ec2[:], AF.Exp, scale=-1.0)
                xmb = scr0.tile([C, sub], BF16, tag="xmb", name="xmb")
                nc.scalar.activation(xmb[:], xm_ps[:], AF.Identity,
                                     bias=conv_b)
                nc.vector.tensor_tensor(xs[:, sl], xmb[:], sgc[:], OP.mult)
                z_ps = ps_b.tile([C, sub], F32, tag="mmb", name="z_ps")
                nc.tensor.matmul(z_ps[:], winT[:, 4 * C:5 * C],
                                 un[:, DCONV + j * sub:
                                     DCONV + j * sub + sub],
                                 start=True, stop=True)
                es1 = scr0.tile([C, sub], F32, tag="es1", name="es1")
                nc.scalar.activation(es1[:], z_ps[:], AF.Exp, scale=-1.0,
                                     bias=nzb_col)
                es2 = scr0.tile([C, sub], F32, tag="es2", name="es2")
                nc.scalar.activation(es2[:], es1[:], AF.Ln, bias=1.0)
                sgz = scr0.tile([C, sub], BF16, tag="sgz", name="sgz")
                nc.scalar.activation(sgz[:], es2[:], AF.Exp, scale=-1.0)
                zmb = scr0.tile([C, sub], BF16, tag="zmb", name="zmb")
                nc.scalar.activation(zmb[:], z_ps[:], AF.Identity,
                                     bias=zb_col)
                nc.vector.tensor_tensor(zs[:, sl], zmb[:], sgz[:], OP.mult)
                dblt = ps_b.tile([C, sub], F32, tag="mmb", name="dblt")
                nc.tensor.matmul(dblt[0:RANK + 2 * NSTATE, :], wxpT[:],
                                 xs[:, sl], start=True, stop=True)
                nc.scalar.copy(bcdt[:, sl], dblt[0:2 * NSTATE + RANK, :])
                nc.sync.dma_start(bcd[0:1, sl],
                                  bcdt[RANK:RANK + 1, sl])
                nc.sync.dma_start(b0[:, sl],
                                  bcd[0:1, sl].broadcast_to([C, sub]))
                dt_ps = ps_b.tile([C, sub], F32, tag="mmb", name="dt_ps")
                nc.tensor.matmul(dt_ps[:], wdtT[:], bcdt[0:RANK, sl],
                                 start=True, stop=True)
                spe = scr0.tile([C, sub], F32, tag="spe", name="spe")
                nc.scalar.activation(spe[:], dt_ps[:], AF.Exp, bias=dt_b)
                nc.scalar.activation(dt_bf[:, sl], spe[:], AF.Ln, bias=1.0)
                nc.scalar.activation(dA0[:, sl], dt_bf[:, sl], AF.Exp,
                                     scale=acol[:, 0:1])
                nc.scalar.activation(dA1[:, sl], dt_bf[:, sl], AF.Exp,
                                     scale=acol[:, 1:2])
                nc.vector.tensor_tensor(v[:, sl], dt_bf[:, sl], xs[:, sl],
                                        OP.mult)
                nc.vector.tensor_tensor(u0[:, sl], v[:, sl], b0[:, sl],
                                        OP.mult)
                nc.vector.tensor_tensor_scan(
                    h0[:, sl], dA0[:, sl], u0[:, sl],
                    0.0 if j == 0 else h0[:, j * sub - 1:j * sub],
                    OP.mult, OP.add)
            d["un"], d["xs"], d["zs"] = un, xs, zs
            d["bcd"], d["dt"], d["v"] = bcd, dt_bf, v
            d["h0"] = h0
            # rest of bcd rows, remaining dA tiles, broadcasts, u_1..u_3
            nc.sync.dma_start(bcd[0:1, Tc:2 * Tc],
                              bcdt[RANK + 1:RANK + 2, :])
            nc.sync.dma_start(bcd[1:NSTATE, :],
                              bcdt[RANK + 2:RANK + 2 * NSTATE, :])
            dAs = [dA0, dA1]
            for n in range(2, NSTATE):
                dA = dap.tile([C, Tc], BF16, tag="dA", name="dA")
                nc.scalar.activation(dA[:], dt_bf[:], AF.Exp,
                                     scale=acol[:, n:n + 1])
                dAs.append(dA)
            d["dAs"] = dAs
            bcdf = bcd.tensor.reshape([1, NSTATE * 2 * Tc])
            prs = []
            for n in range(NSTATE - 1):
                pr = bcrp.tile([C, 2 * Tc], BF16, tag="bcr", name="bcr")
                nc.sync.dma_start(
                    pr[:],
                    bcdf[0:1, (2 * n + 1) * Tc:(2 * n + 3) * Tc]
                    .broadcast_to([C, 2 * Tc]))
                prs.append(pr)
            c15 = b0p.tile([C, Tc], BF16, tag="c15", name="c15")
            nc.sync.dma_start(
                c15[:],
                bcdf[0:1, (2 * NSTATE - 1) * Tc:2 * NSTATE * Tc]
                .broadcast_to([C, Tc]))
            d["prs"], d["c15"], d["b0"] = prs, c15, b0
            d["us"] = [None] * NSTATE
            d["us"][0] = u0
            for n in range(1, 4):
                emit_u(0, n)

        def emit_scan_loop(k):
            d = P[k]
            last = k + 1 >= nchunk
            ysum = ps_y.tile([C, Tc], F32, tag="ysum", name="ysum")
            d["ysum"] = ysum
            for n in range(NSTATE):
                # ---- hooks: pipeline chunk k+1 prefix / chunk k-1 readout
                if n == 1 and k > 0:
                    emit_readout(k - 1)
                if not last:
                    if n == 1:
                        emit_in_dma(k + 1)
                    elif n == 3:
                        emit_ln_a(k + 1)
                    elif n == 6:
                        emit_ln_b(k + 1)
                    elif n == 8:
                        emit_conv(k + 1)
                    elif n == 10:
                        emit_proj(k + 1)
                    elif n == 11:
                        emit_dA(k + 1)
                    elif n == 12:
                        emit_v(k + 1)
                    elif n >= 13:
                        emit_u(k + 1, n - 13)  # u_0..u_2 of k+1

                # ---- chunk k state n
                dA = d["dAs"][n]
                u = d["us"][n]
                init = 0.0 if k == 0 else carries[n][:]
                if k == 0 and n == 0 and d.get("h0") is not None:
                    h = d["h0"]  # pre-scanned in emit_prefix0
                else:
                    h = hp.tile([C, Tc], BF16, tag="h", name="h")
                    nc.vector.tensor_tensor_scan(h[:], dA[:], u[:], init,
                                                 OP.mult, OP.add)
                if not last:
                    nc.vector.tensor_copy(carries[n][:], h[:, Tc - 1:Tc])
                p = pp.tile([C, Tc], BF16, tag="p", name="p")
                cb = d["c15"][:] if n == NSTATE - 1 \
                    else d["prs"][n][:, 0:Tc]
                if n in POOL_P:
                    nc.gpsimd.tensor_tensor(p[:], h[:], cb, OP.mult)
                else:
                    nc.vector.tensor_tensor(p[:], h[:], cb, OP.mult)
                if last and n == NSTATE - 1:
                    # final state: interleave ysum close + readout per sub
                    t0 = k * Tc
                    for j in range(nsub):
                        sl = slice(j * sub, (j + 1) * sub)
                        nc.tensor.matmul(ysum[:, sl], idn[:], p[:, sl],
                                         start=False, stop=False)
                        nc.tensor.matmul(ysum[:, sl], diag_d[:],
                                         d["xs"][:, sl],
                                         start=False, stop=True)
                        yc = ygp.tile([C, sub], BF16, tag="yc", name="yc")
                        nc.scalar.copy(yc[:], ysum[:, sl])
                        yg = ygp.tile([C, sub], BF16, tag="yg", name="yg")
                        nc.vector.tensor_tensor(yg[:], yc[:],
                                                d["zs"][:, sl], OP.mult)
                        o_ps = ps_b.tile([C, sub], F32, tag="mmb",
                                         name="o_ps")
                        nc.tensor.matmul(o_ps[:], woutT[:], yg[:],
                                         start=True, stop=False)
                        nc.tensor.matmul(o_ps[:], idnf[:], d["xin"][:, sl],
                                         start=False, stop=True)
                        ob = ygp.tile([C, sub], F32, tag="ob", name="ob")
                        nc.scalar.copy(ob[:], o_ps[:])
                        nc.sync.dma_start(
                            y_out[:, t0 + j * sub:t0 + (j + 1) * sub],
                            ob[:])
                else:
                    for j in range(nsub):
                        sl = slice(j * sub, (j + 1) * sub)
                        nc.tensor.matmul(ysum[:, sl], idn[:], p[:, sl],
                                         start=(n == 0), stop=False)
                if n + 4 < NSTATE:
                    emit_u(k, n + 4)  # +4 lookahead within chunk k
            if not last:
                emit_u(k + 1, 3)
                for j in range(nsub):
                    sl = slice(j * sub, (j + 1) * sub)
                    nc.tensor.matmul(ysum[:, sl], diag_d[:], d["xs"][:, sl],
                                     start=False, stop=True)

        # ---- bootstrap chunk 0 prefix, then pipelined chunk loop ----
        P[0] = P0
        if True:
            emit_ln_a(0)
            emit_ln_b(0)
            emit_conv(0)
            emit_proj(0)
            emit_dA(0)
            emit_v(0)
            P[0]["h0"] = None
            for n in range(4):
                emit_u(0, n)
        else:
            emit_prefix0()
        for k in range(nchunk):
            emit_scan_loop(k)

    nc.insert_act_table_loads = types.MethodType(_two_act_tables, nc)
    nc.compile()
    return nc


def prep_weights(ln_w, ln_b, in_proj_w, conv_w, conv_b, x_proj_w,
                 dt_proj_w, dt_proj_b, A_log, D, out_proj_w):
    eps = np.full_like(ln_w, LN_EPS)
    z = np.zeros_like(ln_w)
    # fold ln_w into in_proj rows, ln_b into conv/z biases (exact)
    ipx, ipz = in_proj_w[:128], in_proj_w[128:]
    bvec = ipx @ ln_b                    # per-out-channel ln_b feedthrough
    conv_b2 = conv_b + bvec * conv_w.sum(1)
    zb = ipz @ ln_b
    corr = np.stack([-bvec * conv_w[:, :3 - t].sum(1) for t in range(3)],
                    axis=1)              # undo folded bias at t<DCONV-1
    cols = np.stack([ln_w, ln_b, conv_b2, dt_proj_b, D, zb,
                     corr[:, 0], corr[:, 1], corr[:, 2], eps,
                     -conv_b2, -zb, z], axis=1).astype(np.float32)
    return {
        "w_inT": np.ascontiguousarray(np.concatenate(
            [ipx.T * ln_w[:, None] * conv_w[:, kk][None, :]
             for kk in range(4)] + [ipz.T * ln_w[:, None]],
            axis=1).astype(ml_dtypes.bfloat16)),
        "w_xpT": np.ascontiguousarray(
            x_proj_w[list(range(8))
                     + [8 + (i // 2) + 16 * (i % 2) for i in range(32)]].T
            .astype(ml_dtypes.bfloat16)),
        "w_dtT": np.ascontiguousarray(dt_proj_w.T.astype(ml_dtypes.bfloat16)),
        "w_outT": np.ascontiguousarray(
            out_proj_w.T.astype(ml_dtypes.bfloat16)),
        "ident": np.eye(C, dtype=ml_dtypes.bfloat16),
        "nident": (-np.eye(C)).astype(ml_dtypes.bfloat16),
        "diag_d": np.ascontiguousarray(
            np.diag(D).astype(ml_dtypes.bfloat16)),
        "identf": np.eye(C, dtype=np.float32),
        "cols": cols,
        "a_cols": np.ascontiguousarray(-np.exp(A_log.astype(np.float32))),
    }


def kernel(input, ln_w, ln_b, in_proj_w, conv_w, conv_b, x_proj_w,
           dt_proj_w, dt_proj_b, A_log, D, out_proj_w, _run=None):
    input = np.asarray(input, np.float32)
    b, c, H, W = input.shape
    L = H * W
    assert c == C and b == 8
    wts = prep_weights(
        np.asarray(ln_w, np.float32), np.asarray(ln_b, np.float32),
        np.asarray(in_proj_w, np.float32), np.asarray(conv_w, np.float32),
        np.asarray(conv_b, np.float32), np.asarray(x_proj_w, np.float32),
        np.asarray(dt_proj_w, np.float32), np.asarray(dt_proj_b, np.float32),
        np.asarray(A_log, np.float32), np.asarray(D, np.float32),
        np.asarray(out_proj_w, np.float32))
    nc = build_nc(L, 1536, 512)
    in_maps = []
    for i in range(8):
        m = {"x": np.ascontiguousarray(input[i].reshape(c, L))}
        m.update(wts)
        in_maps.append(m)
    run = _run or run_bass_kernel_spmd
    res = run(nc, in_maps, core_ids=list(range(8)))
    out = np.stack([np.asarray(res.results[i]["y"]).reshape(c, H, W)
                    for i in range(8)])
    return out.astype(np.float32)


# revision 3
# speedup vs baseline: 1.0119x; 1.0029x over previous
"""CAMMambaBlock Trainium2 kernel, v2.

Data-parallel over batch: 8 batch elements -> 8 NeuronCores. Each core runs
the full block on its (c=128, L=9216) slice, streaming over L in 6 chunks of
1536.

Key structure: the per-state mults are UNFUSED from the scan chain.
u_n = v*B_n depends only on prefix outputs, so all 16 scans run
back-to-back on DVE with no interleaved serial mults; p_n = h_n*C_n runs
after scan_n and feeds PE identity-matmul ysum accumulation. Silu/softplus
use direct activation functions (2 act-table swaps per chunk); ln_w/ln_b
are folded into the in_proj weights/biases host-side so LN is two DVE ops.
Cross-chunk software pipelining via emission hooks inside the scan loop.
"""
import types
import numpy as np
import ml_dtypes
from contextlib import ExitStack

import bass_rust

import concourse.bass as bass
import concourse.bacc as bacc
import concourse.tile as tile
from concourse import mybir
from concourse.bass_utils import run_bass_kernel_spmd
from concourse.hw_specs import get_activation_tables


def _two_act_tables(self):
    """Limit activation tables to the two we batch by, so the table-load
    pass inserts at most one swap per batch."""
    if not any(i.opcode == "Activation" for i in self.all_instructions()):
        return
    keep = ("natural_log_exp_and_others", "silu_and_others")
    tables = [(n, (f if n in keep else set()))
              for n, f in get_activation_tables(self.m.arch).items()]
    bass_rust.insert_act_table_loads(self, tables)

F32 = mybir.dt.float32
BF16 = mybir.dt.bfloat16
AF = mybir.ActivationFunctionType
OP = mybir.AluOpType

C = 128
NSTATE = 16
RANK = 8
LN_EPS = 1e-5
DCONV = 4

L_FULL = 96 * 96

# states whose p-mult runs on the Pool engine. Measured: Pool shares an SBUF
# port with DVE 2-port ops, so Pool compute during scans halves BOTH engines'
# throughput. Keep empty.
POOL_P = ()


def build_nc(L, Tc, sub=512):
    assert L % Tc == 0 and Tc % sub == 0
    nchunk = L // Tc
    nsub = Tc // sub

    nc = bacc.Bacc()
    x_in = nc.declare_dram_parameter("x", [C, L], F32, isOutput=False)
    w_inT = nc.declare_dram_parameter("w_inT", [C, 5 * C], BF16, isOutput=False)
    w_xpT = nc.declare_dram_parameter("w_xpT", [C, RANK + 2 * NSTATE], BF16,
                                      isOutput=False)
    w_dtT = nc.declare_dram_parameter("w_dtT", [RANK, C], BF16, isOutput=False)
    w_outT = nc.declare_dram_parameter("w_outT", [C, C], BF16, isOutput=False)
    ident = nc.declare_dram_parameter("ident", [C, C], BF16, isOutput=False)
    nident = nc.declare_dram_parameter("nident", [C, C], BF16, isOutput=False)
    diag_d_in = nc.declare_dram_parameter("diag_d", [C, C], BF16,
                                          isOutput=False)
    identf = nc.declare_dram_parameter("identf", [C, C], F32, isOutput=False)
    # per-partition columns:
    # [ln_w, ln_b, conv_b, dt_b, D, unused*4, eps, -conv_b]
    cols = nc.declare_dram_parameter("cols", [C, 13], F32, isOutput=False)
    a_cols = nc.declare_dram_parameter("a_cols", [C, NSTATE], F32,
                                       isOutput=False)
    y_out = nc.declare_dram_parameter("y", [C, L], F32, isOutput=True)

    with tile.TileContext(nc) as tc, ExitStack() as ctx:
        wpool = ctx.enter_context(tc.tile_pool(name="weights", bufs=1))
        io = ctx.enter_context(tc.tile_pool(name="io", bufs=2))
        sqp = ctx.enter_context(tc.tile_pool(name="sqp", bufs=1))
        lnp = ctx.enter_context(tc.tile_pool(name="lnp", bufs=1))
        unp = ctx.enter_context(tc.tile_pool(name="unp", bufs=2))
        gate = ctx.enter_context(tc.tile_pool(name="gate", bufs=2))
        dtvp = ctx.enter_context(tc.tile_pool(name="dtvp", bufs=2))
        bctp = ctx.enter_context(tc.tile_pool(name="bctp", bufs=2))
        scr = ctx.enter_context(tc.tile_pool(name="scr", bufs=2))
        scr0 = ctx.enter_context(tc.tile_pool(name="scr0", bufs=1))
        dap = ctx.enter_context(tc.tile_pool(name="dap", bufs=9))
        bcrp = ctx.enter_context(tc.tile_pool(name="bcrp", bufs=6))
        b0p = ctx.enter_context(tc.tile_pool(name="b0p", bufs=2))
        up = ctx.enter_context(tc.tile_pool(name="up", bufs=4))
        hp = ctx.enter_context(tc.tile_pool(name="hp", bufs=3))
        pp = ctx.enter_context(tc.tile_pool(name="pp", bufs=3))
        ygp = ctx.enter_context(tc.tile_pool(name="ygp", bufs=2))
        state = ctx.enter_context(tc.tile_pool(name="state", bufs=1))
        dram = ctx.enter_context(tc.tile_pool(name="dram", bufs=2,
                                              space="DRAM"))
        ps_st = ctx.enter_context(tc.tile_pool(name="ps_st", bufs=1,
                                               space="PSUM"))
        ps_b = ctx.enter_context(tc.tile_pool(name="ps_b", bufs=2,
                                              space="PSUM"))
        ps_y = ctx.enter_context(tc.tile_pool(name="ps_y", bufs=1,
                                              space="PSUM"))

        # ---- chunk-0 input first: nothing else gates the LN stats ----
        xin0 = io.tile([C, Tc], F32, tag="xin", name="xin0")
        nc.sync.dma_start(xin0[:], x_in[:, 0:Tc])
        xinbf0 = io.tile([C, Tc], BF16, tag="xinbf", name="xinbf0")
        nc.gpsimd.dma_start(xinbf0[:], x_in[:, 0:Tc])
        P0 = {"xin": xin0, "xinbf": xinbf0}

        # ---- weights ----
        ones_c = wpool.tile([C, C], BF16, tag="ones")
        nc.gpsimd.memset(ones_c[:], 1.0 / C)
        winT = wpool.tile([C, 5 * C], BF16, tag="winT")
        nc.sync.dma_start(winT[:], w_inT[:])
        wxpT = wpool.tile([C, RANK + 2 * NSTATE], BF16, tag="wxpT")
        nc.sync.dma_start(wxpT[:], w_xpT[:])
        wdtT = wpool.tile([RANK, C], BF16, tag="wdtT")
        nc.sync.dma_start(wdtT[:], w_dtT[:])
        woutT = wpool.tile([C, C], BF16, tag="woutT")
        nc.sync.dma_start(woutT[:], w_outT[:])
        idn = wpool.tile([C, C], BF16, tag="idn")
        nc.sync.dma_start(idn[:], ident[:])
        nidn = wpool.tile([C, C], BF16, tag="nidn")
        nc.sync.dma_start(nidn[:], nident[:])
        diag_d = wpool.tile([C, C], BF16, tag="diag_d")
        nc.sync.dma_start(diag_d[:], diag_d_in[:])
        idnf = wpool.tile([C, C], F32, tag="idnf")
        nc.sync.dma_start(idnf[:], identf[:])
        colsb = wpool.tile([C, 13], F32, tag="cols")
        nc.sync.dma_start(colsb[:], cols[:])
        acol = wpool.tile([C, NSTATE], F32, tag="acol")
        nc.sync.dma_start(acol[:], a_cols[:])
        conv_b, dt_b = colsb[:, 2:3], colsb[:, 3:4]
        zb_col = colsb[:, 5:6]
        corr_cols = colsb[:, 6:9]
        eps_col = colsb[:, 9:10]
        ncb_col = colsb[:, 10:11]
        nzb_col = colsb[:, 11:12]

        carries = [state.tile([C, 1], BF16, tag=f"carry{n}",
                              name=f"carry{n}")
                   for n in range(NSTATE)]

        P = {}  # per-chunk produced tiles

        def emit_in_dma(k):
            t0 = k * Tc
            d = P.setdefault(k, {})
            d["xin"] = io.tile([C, Tc], F32, tag="xin", name="xin")
            nc.sync.dma_start(d["xin"][:], x_in[:, t0:t0 + Tc])
            d["xinbf"] = io.tile([C, Tc], BF16, tag="xinbf", name="xinbf")
            nc.gpsimd.dma_start(d["xinbf"][:], x_in[:, t0:t0 + Tc])

        def emit_ln_a(k):
            d = P[k]
            sq = sqp.tile([C, Tc], BF16, tag="sq", name="sq")
            nc.scalar.activation(sq[:], d["xin"][:], AF.Square)
            mub = lnp.tile([C, Tc], BF16, tag="mub", name="mub")
            rstd = lnp.tile([C, Tc], BF16, tag="rstd", name="rstd")
            for j in range(nsub):
                sl = slice(j * sub, (j + 1) * sub)
                mu = ps_st.tile([C, sub], F32, tag="mu", name="mu")
                nc.tensor.matmul(mu[:], ones_c[:], d["xinbf"][:, sl],
                                 start=True, stop=True)
                musq = scr.tile([C, sub], BF16, tag="musq", name="musq")
                nc.scalar.activation(musq[:], mu[:], AF.Square)
                nc.scalar.copy(mub[:, sl], mu[:])
                var = ps_st.tile([C, sub], F32, tag="m2", name="var")
                nc.tensor.matmul(var[:], ones_c[:], sq[:, sl],
                                 start=True, stop=False)
                nc.tensor.matmul(var[:], nidn[:], musq[:],
                                 start=False, stop=True)
                lnv = scr0.tile([C, sub], F32, tag="lnv", name="lnv")
                nc.scalar.activation(lnv[:], var[:], AF.Ln, bias=eps_col)
                nc.scalar.activation(rstd[:, sl], lnv[:], AF.Exp, scale=-0.5)
            d["mub"], d["rstd"] = mub, rstd

        def emit_ln_b(k):
            d = P[k]
            # ln_w is folded into the in_proj weights and ln_b into the
            # conv/z biases (host-side), so un is just (x-mu)*rstd. Halo
            # lives at cols 1..3; data at 4..Tc+3 (4B-aligned write).
            un = unp.tile([C, Tc + DCONV], BF16, tag="un", name="un")
            if k == 0:
                nc.vector.memset(un[:, 1:DCONV], 0.0)
            else:
                nc.vector.tensor_copy(un[:, 1:DCONV],
                                      P[k - 1]["un"][:, Tc + 1:Tc + DCONV])
            if k == 0:
                for j in range(nsub):
                    sl = slice(j * sub, (j + 1) * sub)
                    dmu = scr.tile([C, sub], BF16, tag="dmu0", name="dmu")
                    nc.vector.tensor_tensor(dmu[:], d["xinbf"][:, sl],
                                            d["mub"][:, sl], OP.subtract)
                    nc.vector.tensor_tensor(
                        un[:, DCONV + j * sub:DCONV + (j + 1) * sub],
                        dmu[:], d["rstd"][:, sl], OP.mult)
            else:
                dmu = scr.tile([C, Tc], BF16, tag="dmu", name="dmu")
                nc.vector.tensor_tensor(dmu[:], d["xinbf"][:], d["mub"][:],
                                        OP.subtract)
                nc.vector.tensor_tensor(un[:, DCONV:Tc + DCONV],
                                        dmu[:], d["rstd"][:], OP.mult)
            d["un"] = un

        def emit_conv(k):
            d = P[k]
            un = d["un"]
            xs = gate.tile([C, Tc], BF16, tag="xs", name="xs")
            zs = gate.tile([C, Tc], BF16, tag="zs", name="zs")
            for j in range(nsub):
                sl = slice(j * sub, (j + 1) * sub)
                xm_ps = ps_b.tile([C, sub], F32, tag="mmb", name="xm_ps")
                for kk in range(DCONV):
                    nc.tensor.matmul(
                        xm_ps[:], winT[:, kk * C:(kk + 1) * C],
                        un[:, kk + 1 + j * sub:kk + 1 + j * sub + sub],
                        start=(kk == 0), stop=(kk == DCONV - 1))
                nc.scalar.activation(xs[:, sl], xm_ps[:], AF.Silu,
                                     bias=conv_b)
            for j in range(nsub):
                sl = slice(j * sub, (j + 1) * sub)
                z_ps = ps_b.tile([C, sub], F32, tag="mmb", name="z_ps")
                nc.tensor.matmul(z_ps[:], winT[:, 4 * C:5 * C],
                                 un[:, DCONV + j * sub:
                                     DCONV + j * sub + sub],
                                 start=True, stop=True)
                nc.scalar.activation(zs[:, sl], z_ps[:], AF.Silu,
                                     bias=zb_col)
            d["xs"], d["zs"] = xs, zs

        def emit_proj(k):
            d = P[k]
            xs = d["xs"]
            bcdt = bctp.tile([2 * NSTATE + RANK, Tc], BF16, tag="bcdt",
                             name="bcdt")
            for j in range(nsub):
                sl = slice(j * sub, (j + 1) * sub)
                dblt = ps_b.tile([C, sub], F32, tag="mmb", name="dblt")
                nc.tensor.matmul(dblt[0:RANK + 2 * NSTATE, :], wxpT[:],
                                 xs[:, sl], start=True, stop=True)
                nc.scalar.copy(bcdt[:, sl], dblt[0:2 * NSTATE + RANK, :])
            bcd = dram.tile([NSTATE, 2 * Tc], BF16, tag="bcd", name="bcd")
            # B_0 per-sub first so the first u-mult can start early
            for j in range(nsub):
                nc.sync.dma_start(bcd[0:1, j * sub:(j + 1) * sub],
                                  bcdt[RANK:RANK + 1, j * sub:(j + 1) * sub])
            nc.sync.dma_start(bcd[0:1, Tc:2 * Tc],
                              bcdt[RANK + 1:RANK + 2, :])
            nc.sync.dma_start(bcd[1:NSTATE, :],
                              bcdt[RANK + 2:RANK + 2 * NSTATE, :])
            d["bcd"] = bcd

            dt_bf = dtvp.tile([C, Tc], BF16, tag="dt", name="dt")
            for j in range(nsub):
                sl = slice(j * sub, (j + 1) * sub)
                dt_ps = ps_b.tile([C, sub], F32, tag="mmb", name="dt_ps")
                nc.tensor.matmul(dt_ps[:], wdtT[:], bcdt[0:RANK, sl],
                                 start=True, stop=True)
                spe = scr0.tile([C, sub], F32, tag="spe", name="spe")
                nc.scalar.activation(spe[:], dt_ps[:], AF.Exp, bias=dt_b)
                nc.scalar.activation(dt_bf[:, sl], spe[:], AF.Ln, bias=1.0)
            d["dt"] = dt_bf

        def emit_dA(k):
            d = P[k]
            dAs = []
            for n in range(NSTATE):
                dA = dap.tile([C, Tc], BF16, tag="dA", name="dA")
                nc.scalar.activation(dA[:], d["dt"][:], AF.Exp,
                                     scale=acol[:, n:n + 1])
                dAs.append(dA)
            d["dAs"] = dAs
            # broadcast DMAs: B_0, then pairs (C_n | B_{n+1}), then C_15
            bcd = d["bcd"]
            bcdf = bcd.tensor.reshape([1, NSTATE * 2 * Tc])
            b0 = b0p.tile([C, Tc], BF16, tag="b0", name="b0")
            for j in range(nsub):
                sl = slice(j * sub, (j + 1) * sub)
                nc.sync.dma_start(b0[:, sl],
                                  bcd[0:1, sl].broadcast_to([C, sub]))
            d["b0"] = b0
            prs = []
            for n in range(NSTATE - 1):
                pr = bcrp.tile([C, 2 * Tc], BF16, tag="bcr", name="bcr")
                nc.sync.dma_start(
                    pr[:],
                    bcdf[0:1, (2 * n + 1) * Tc:(2 * n + 3) * Tc]
                    .broadcast_to([C, 2 * Tc]))
                prs.append(pr)
            c15 = b0p.tile([C, Tc], BF16, tag="c15", name="c15")
            nc.sync.dma_start(
                c15[:],
                bcdf[0:1, (2 * NSTATE - 1) * Tc:2 * NSTATE * Tc]
                .broadcast_to([C, Tc]))
            d["prs"], d["c15"] = prs, c15

        def emit_v(k):
            d = P[k]
            v = dtvp.tile([C, Tc], BF16, tag="v", name="v")
            nc.vector.tensor_tensor(v[:], d["dt"][:], d["xs"][:], OP.mult)
            d["v"] = v
            d["us"] = [None] * NSTATE

        def emit_u(k, n):
            d = P[k]
            u = up.tile([C, Tc], BF16, tag="u", name="u")
            if n == 0:
                nc.vector.tensor_tensor(u[:], d["v"][:], d["b0"][:], OP.mult)
            else:
                nc.vector.tensor_tensor(u[:], d["v"][:],
                                        d["prs"][n - 1][:, Tc:2 * Tc],
                                        OP.mult)
            d["us"][n] = u

        def emit_readout(k):
            d = P[k]
            t0 = k * Tc
            ysum, zs, xin = d["ysum"], d["zs"], d["xin"]
            for j in range(nsub):
                sl = slice(j * sub, (j + 1) * sub)
                yc = ygp.tile([C, sub], BF16, tag="yc", name="yc")
                nc.scalar.copy(yc[:], ysum[:, sl])
                yg = ygp.tile([C, sub], BF16, tag="yg", name="yg")
                nc.vector.tensor_tensor(yg[:], yc[:], zs[:, sl], OP.mult)
                o_ps = ps_b.tile([C, sub], F32, tag="mmb", name="o_ps")
                nc.tensor.matmul(o_ps[:], woutT[:], yg[:],
                                 start=True, stop=False)
                nc.tensor.matmul(o_ps[:], idnf[:], xin[:, sl],
                                 start=False, stop=True)
                ob = ygp.tile([C, sub], F32, tag="ob", name="ob")
                nc.scalar.copy(ob[:], o_ps[:])
                nc.sync.dma_start(y_out[:, t0 + j * sub:t0 + (j + 1) * sub],
                                  ob[:])

        def emit_prefix0():
            """Chunk-0 prefix, per-sub pipelined so the first scan starts
            ~25us earlier. Silu synthesized from exp/ln (no table swap in
            the ramp). State-0's scan runs as chained sub-scans emitted
            inline; states 0/1 dA and u_0 are produced per-sub."""
            d = P[0]
            sq = sqp.tile([C, Tc], BF16, tag="sq", name="sq")
            mub = lnp.tile([C, Tc], BF16, tag="mub", name="mub")
            rstd = lnp.tile([C, Tc], BF16, tag="rstd", name="rstd")
            un = unp.tile([C, Tc + DCONV], BF16, tag="un", name="un")
            nc.vector.memset(un[:, 1:DCONV], 0.0)
            xs = gate.tile([C, Tc], BF16, tag="xs", name="xs")
            zs = gate.tile([C, Tc], BF16, tag="zs", name="zs")
            bcdt = bctp.tile([2 * NSTATE + RANK, Tc], BF16, tag="bcdt",
                             name="bcdt")
            bcd = dram.tile([NSTATE, 2 * Tc], BF16, tag="bcd", name="bcd")
            dt_bf = dtvp.tile([C, Tc], BF16, tag="dt", name="dt")
            v = dtvp.tile([C, Tc], BF16, tag="v", name="v")
            dA0 = dap.tile([C, Tc], BF16, tag="dA", name="dA0")
            dA1 = dap.tile([C, Tc], BF16, tag="dA", name="dA1")
            b0 = b0p.tile([C, Tc], BF16, tag="b0", name="b0")
            u0 = up.tile([C, Tc], BF16, tag="u", name="u0")
            h0 = hp.tile([C, Tc], BF16, tag="h", name="h0")
            for j in range(nsub):
                sl = slice(j * sub, (j + 1) * sub)
                nc.scalar.activation(sq[:, sl], d["xin"][:, sl], AF.Square)
                mu = ps_st.tile([C, sub], F32, tag="mu", name="mu")
                nc.tensor.matmul(mu[:], ones_c[:], d["xinbf"][:, sl],
                                 start=True, stop=True)
                musq = scr.tile([C, sub], BF16, tag="musq", name="musq")
                nc.scalar.activation(musq[:], mu[:], AF.Square)
                nc.scalar.copy(mub[:, sl], mu[:])
                var = ps_st.tile([C, sub], F32, tag="m2", name="var")
                nc.tensor.matmul(var[:], ones_c[:], sq[:, sl],
                                 start=True, stop=False)
                nc.tensor.matmul(var[:], nidn[:], musq[:],
                                 start=False, stop=True)
                lnv = scr0.tile([C, sub], F32, tag="lnv", name="lnv")
                nc.scalar.activation(lnv[:], var[:], AF.Ln, bias=eps_col)
                nc.scalar.activation(rstd[:, sl], lnv[:], AF.Exp, scale=-0.5)
                dmu = scr.tile([C, sub], BF16, tag="dmu", name="dmu")
                nc.vector.tensor_tensor(dmu[:], d["xinbf"][:, sl],
                                        mub[:, sl], OP.subtract)
                nc.vector.tensor_tensor(
                    un[:, DCONV + j * sub:DCONV + (j + 1) * sub],
                    dmu[:], rstd[:, sl], OP.mult)
                xm_ps = ps_b.tile([C, sub], F32, tag="mmb", name="xm_ps")
                for kk in range(DCONV):
                    nc.tensor.matmul(
                        xm_ps[:], winT[:, kk * C:(kk + 1) * C],
                        un[:, kk + 1 + j * sub:kk + 1 + j * sub + sub],
                        start=(kk == 0), stop=(kk == DCONV - 1))
                ec1 = scr0.tile([C, sub], F32, tag="ec1", name="ec1")
                nc.scalar.activation(ec1[:], xm_ps[:], AF.Exp, scale=-1.0,
                                     bias=ncb_col)
                ec2 = scr0.tile([C, sub], F32, tag="ec2", name="ec2")
                nc.scalar.activation(ec2[:], ec1[:], AF.Ln, bias=1.0)
                sgc = scr0.tile([C, sub], BF16, tag="sgc", name="sgc")
                nc.scalar.activation(sgc[:], ec2[:], AF.Exp, scale=-1.0)
                xmb = scr0.tile([C, sub], BF16, tag="xmb", name="xmb")
                nc.scalar.activation(xmb[:], xm_ps[:], AF.Identity,
                                     bias=conv_b)
                nc.vector.tensor_tensor(xs[:, sl], xmb[:], sgc[:], OP.mult)
                z_ps = ps_b.tile([C, sub], F32, tag="mmb", name="z_ps")
                nc.tensor.matmul(z_ps[:], winT[:, 4 * C:5 * C],
                                 un[:, DCONV + j * sub:
                                     DCONV + j * sub + sub],
                                 start=True, stop=True)
                es1 = scr0.tile([C, sub], F32, tag="es1", name="es1")
                nc.scalar.activation(es1[:], z_ps[:], AF.Exp, scale=-1.0,
                                     bias=nzb_col)
                es2 = scr0.tile([C, sub], F32, tag="es2", name="es2")
                nc.scalar.activation(es2[:], es1[:], AF.Ln, bias=1.0)
                sgz = scr0.tile([C, sub], BF16, tag="sgz", name="sgz")
                nc.scalar.activation(sgz[:], es2[:], AF.Exp, scale=-1.0)
                zmb = scr0.tile([C, sub], BF16, tag="zmb", name="zmb")
                nc.scalar.activation(zmb[:], z_ps[:], AF.Identity,
                                     bias=zb_col)
                nc.vector.tensor_tensor(zs[:, sl], zmb[:], sgz[:], OP.mult)
                dblt = ps_b.tile([C, sub], F32, tag="mmb", name="dblt")
                nc.tensor.matmul(dblt[0:RANK + 2 * NSTATE, :], wxpT[:],
                                 xs[:, sl], start=True, stop=True)
                nc.scalar.copy(bcdt[:, sl], dblt[0:2 * NSTATE + RANK, :])
                nc.sync.dma_start(bcd[0:1, sl],
                                  bcdt[RANK:RANK + 1, sl])
                nc.sync.dma_start(b0[:, sl],
                                  bcd[0:1, sl].broadcast_to([C, sub]))
                dt_ps = ps_b.tile([C, sub], F32, tag="mmb", name="dt_ps")
                nc.tensor.matmul(dt_ps[:], wdtT[:], bcdt[0:RANK, sl],
                                 start=True, stop=True)
                spe = scr0.tile([C, sub], F32, tag="spe", name="spe")
                nc.scalar.activation(spe[:], dt_ps[:], AF.Exp, bias=dt_b)
                nc.scalar.activation(dt_bf[:, sl], spe[:], AF.Ln, bias=1.0)
                nc.scalar.activation(dA0[:, sl], dt_bf[:, sl], AF.Exp,
                                     scale=acol[:, 0:1])
                nc.scalar.activation(dA1[:, sl], dt_bf[:, sl], AF.Exp,
                                     scale=acol[:, 1:2])
                nc.vector.tensor_tensor(v[:, sl], dt_bf[:, sl], xs[:, sl],
                                        OP.mult)
                nc.vector.tensor_tensor(u0[:, sl], v[:, sl], b0[:, sl],
                                        OP.mult)
                nc.vector.tensor_tensor_scan(
                    h0[:, sl], dA0[:, sl], u0[:, sl],
                    0.0 if j == 0 else h0[:, j * sub - 1:j * sub],
                    OP.mult, OP.add)
            d["un"], d["xs"], d["zs"] = un, xs, zs
            d["bcd"], d["dt"], d["v"] = bcd, dt_bf, v
            d["h0"] = h0
            # rest of bcd rows, remaining dA tiles, broadcasts, u_1..u_3
            nc.sync.dma_start(bcd[0:1, Tc:2 * Tc],
                              bcdt[RANK + 1:RANK + 2, :])
            nc.sync.dma_start(bcd[1:NSTATE, :],
                              bcdt[RANK + 2:RANK + 2 * NSTATE, :])
            dAs = [dA0, dA1]
            for n in range(2, NSTATE):
                dA = dap.tile([C, Tc], BF16, tag="dA", name="dA")
                nc.scalar.activation(dA[:], dt_bf[:], AF.Exp,
                                     scale=acol[:, n:n + 1])
                dAs.append(dA)
            d["dAs"] = dAs
            bcdf = bcd.tensor.reshape([1, NSTATE * 2 * Tc])
            prs = []
            for n in range(NSTATE - 1):
                pr = bcrp.tile([C, 2 * Tc], BF16, tag="bcr", name="bcr")
                nc.sync.dma_start(
                    pr[:],
                    bcdf[0:1, (2 * n + 1) * Tc:(2 * n + 3) * Tc]
                    .broadcast_to([C, 2 * Tc]))
                prs.append(pr)
            c15 = b0p.tile([C, Tc], BF16, tag="c15", name="c15")
            nc.sync.dma_start(
                c15[:],
                bcdf[0:1, (2 * NSTATE - 1) * Tc:2 * NSTATE * Tc]
                .broadcast_to([C, Tc]))
            d["prs"], d["c15"], d["b0"] = prs, c15, b0
            d["us"] = [None] * NSTATE
            d["us"][0] = u0
            for n in range(1, 4):
                emit_u(0, n)

        def emit_scan_loop(k):
            d = P[k]
            last = k + 1 >= nchunk
            ysum = ps_y.tile([C, Tc], F32, tag="ysum", name="ysum")
            d["ysum"] = ysum
            for n in range(NSTATE):
                # ---- hooks: pipeline chunk k+1 prefix / chunk k-1 readout
                if n == 1 and k > 0:
                    emit_readout(k - 1)
                if not last:
                    if n == 1:
                        emit_in_dma(k + 1)
                    elif n == 3:
                        emit_ln_a(k + 1)
                    elif n == 6:
                        emit_ln_b(k + 1)
                    elif n == 8:
                        emit_conv(k + 1)
                    elif n == 10:
                        emit_proj(k + 1)
                    elif n == 11:
                        emit_dA(k + 1)
                    elif n == 12:
                        emit_v(k + 1)
                    elif n >= 13:
                        emit_u(k + 1, n - 13)  # u_0..u_2 of k+1

                # ---- chunk k state n
                dA = d["dAs"][n]
                u = d["us"][n]
                init = 0.0 if k == 0 else carries[n][:]
                if last and n == NSTATE - 1:
                    # final state: chained sub-scans so the readout of sub j
                    # overlaps the scan of sub j+1
                    t0 = k * Tc
                    h = hp.tile([C, Tc], BF16, tag="h", name="h")
                    p = pp.tile([C, Tc], BF16, tag="p", name="p")
                    cb = d["c15"]
                    for j in range(nsub):
                        sl = slice(j * sub, (j + 1) * sub)
                        nc.vector.tensor_tensor_scan(
                            h[:, sl], dA[:, sl], u[:, sl],
                            init if j == 0 else h[:, j * sub - 1:j * sub],
                            OP.mult, OP.add)
                        nc.vector.tensor_tensor(p[:, sl], h[:, sl],
                                                cb[:, sl], OP.mult)
                        nc.tensor.matmul(ysum[:, sl], idn[:], p[:, sl],
                                         start=False, stop=False)
                        nc.tensor.matmul(ysum[:, sl], diag_d[:],
                                         d["xs"][:, sl],
                                         start=False, stop=True)
                        yc = ygp.tile([C, sub], BF16, tag="yc", name="yc")
                        nc.scalar.copy(yc[:], ysum[:, sl])
                        yg = ygp.tile([C, sub], BF16, tag="yg", name="yg")
                        nc.vector.tensor_tensor(yg[:], yc[:],
                                                d["zs"][:, sl], OP.mult)
                        o_ps = ps_b.tile([C, sub], F32, tag="mmb",
                                         name="o_ps")
                        nc.tensor.matmul(o_ps[:], woutT[:], yg[:],
                                         start=True, stop=False)
                        nc.tensor.matmul(o_ps[:], idnf[:], d["xin"][:, sl],
                                         start=False, stop=True)
                        ob = ygp.tile([C, sub], F32, tag="ob", name="ob")
                        nc.scalar.copy(ob[:], o_ps[:])
                        nc.sync.dma_start(
                            y_out[:, t0 + j * sub:t0 + (j + 1) * sub],
                            ob[:])
                else:
                    h = hp.tile([C, Tc], BF16, tag="h", name="h")
                    nc.vector.tensor_tensor_scan(h[:], dA[:], u[:], init,
                                                 OP.mult, OP.add)
                    if not last:
                        nc.vector.tensor_copy(carries[n][:],
                                              h[:, Tc - 1:Tc])
                    p = pp.tile([C, Tc], BF16, tag="p", name="p")
                    cb = d["c15"][:] if n == NSTATE - 1 \
                        else d["prs"][n][:, 0:Tc]
                    nc.vector.tensor_tensor(p[:], h[:], cb, OP.mult)
                    for j in range(nsub):
                        sl = slice(j * sub, (j + 1) * sub)
                        nc.tensor.matmul(ysum[:, sl], idn[:], p[:, sl],
                                         start=(n == 0), stop=False)
                if n + 4 < NSTATE:
                    emit_u(k, n + 4)  # +4 lookahead within chunk k
            if not last:
                emit_u(k + 1, 3)
                for j in range(nsub):
                    sl = slice(j * sub, (j + 1) * sub)
                    nc.tensor.matmul(ysum[:, sl], diag_d[:], d["xs"][:, sl],
                                     start=False, stop=True)

        # ---- bootstrap chunk 0 prefix, then pipelined chunk loop ----
        P[0] = P0
        if True:
            emit_ln_a(0)
            emit_ln_b(0)
            emit_conv(0)
            emit_proj(0)
            emit_dA(0)
            emit_v(0)
            P[0]["h0"] = None
            for n in range(4):
                emit_u(0, n)
        else:
            emit_prefix0()
        for k in range(nchunk):
            emit_scan_loop(k)

    nc.insert_act_table_loads = types.MethodType(_two_act_tables, nc)
    nc.compile()
    return nc


def prep_weights(ln_w, ln_b, in_proj_w, conv_w, conv_b, x_proj_w,
                 dt_proj_w, dt_proj_b, A_log, D, out_proj_w):
    eps = np.full_like(ln_w, LN_EPS)
    z = np.zeros_like(ln_w)
    # fold ln_w into in_proj rows, ln_b into conv/z biases (exact)
    ipx, ipz = in_proj_w[:128], in_proj_w[128:]
    bvec = ipx @ ln_b                    # per-out-channel ln_b feedthrough
    conv_b2 = conv_b + bvec * conv_w.sum(1)
    zb = ipz @ ln_b
    corr = np.stack([-bvec * conv_w[:, :3 - t].sum(1) for t in range(3)],
                    axis=1)              # undo folded bias at t<DCONV-1
    cols = np.stack([ln_w, ln_b, conv_b2, dt_proj_b, D, zb,
                     corr[:, 0], corr[:, 1], corr[:, 2], eps,
                     -conv_b2, -zb, z], axis=1).astype(np.float32)
    return {
        "w_inT": np.ascontiguousarray(np.concatenate(
            [ipx.T * ln_w[:, None] * conv_w[:, kk][None, :]
             for kk in range(4)] + [ipz.T * ln_w[:, None]],
            axis=1).astype(ml_dtypes.bfloat16)),
        "w_xpT": np.ascontiguousarray(
            x_proj_w[list(range(8))
                     + [8 + (i // 2) + 16 * (i % 2) for i in range(32)]].T
            .astype(ml_dtypes.bfloat16)),
        "w_dtT": np.ascontiguousarray(dt_proj_w.T.astype(ml_dtypes.bfloat16)),
        "w_outT": np.ascontiguousarray(
            out_proj_w.T.astype(ml_dtypes.bfloat16)),
        "ident": np.eye(C, dtype=ml_dtypes.bfloat16),
        "nident": (-np.eye(C)).astype(ml_dtypes.bfloat16),
        "diag_d": np.ascontiguousarray(
            np.diag(D).astype(ml_dtypes.bfloat16)),
        "identf": np.eye(C, dtype=np.float32),
        "cols": cols,
        "a_cols": np.ascontiguousarray(-np.exp(A_log.astype(np.float32))),
    }


def kernel(input, ln_w, ln_b, in_proj_w, conv_w, conv_b, x_proj_w,
           dt_proj_w, dt_proj_b, A_log, D, out_proj_w, _run=None):
    input = np.asarray(input, np.float32)
    b, c, H, W = input.shape
    L = H * W
    assert c == C and b == 8
    wts = prep_weights(
        np.asarray(ln_w, np.float32), np.asarray(ln_b, np.float32),
        np.asarray(in_proj_w, np.float32), np.asarray(conv_w, np.float32),
        np.asarray(conv_b, np.float32), np.asarray(x_proj_w, np.float32),
        np.asarray(dt_proj_w, np.float32), np.asarray(dt_proj_b, np.float32),
        np.asarray(A_log, np.float32), np.asarray(D, np.float32),
        np.asarray(out_proj_w, np.float32))
    nc = build_nc(L, 1536, 512)
    in_maps = []
    for i in range(8):
        m = {"x": np.ascontiguousarray(input[i].reshape(c, L))}
        m.update(wts)
        in_maps.append(m)
    run = _run or run_bass_kernel_spmd
    res = run(nc, in_maps, core_ids=list(range(8)))
    out = np.stack([np.asarray(res.results[i]["y"]).reshape(c, H, W)
                    for i in range(8)])
    return out.astype(np.float32)
